# revision 2
# baseline (speedup 1.0000x reference)
"""Trainium2 Bass kernel for nn_DiscreteAutoregressiveFlow (sampling, forward).

Math: `inputs` is an exact one-hot [B, L, V] tensor. For a row holding token v:
  net = W[v] + b                      (exact: one-hot @ W picks a row)
  loc = one_hot(argmax(net[:V]));  scale = one_hot(argmax(net[V:]))
  one_hot_multiply -> one-hot at (scale_tok*v) % V   (zero row if scale_tok==0)
  one_hot_add      -> one-hot at (scale_tok*v + loc_tok) % V
So out[row] = one_hot(cmap[v]) with a host-precomputed 64-entry map
(sentinel >= V encodes the zero row). The straight-through softmax residuals
and FFT noise in the reference are O(1e-7) and vanish in norm relative error.

Device pipeline per 128x(r*64) chunk (pure streaming, memory-bound), all f32:
  prefill = cmap/128 broadcast into the chunk tile   (scalar ACT copy)
  prod    = accum-DMA x into prefill                  (SWDGE CCE add: x + cmap)
  m       = reduce_max(prod, inner V)                 (DVE) = 1 + cmap[tok]/128
  out     = is_equal(1 + iota/128, m)                 (DVE) -> exact 0.0/1.0
  DMA-out (sync HWDGE)
The accum-DMA fuses the gather-add into the transfer so DVE runs only two
passes (reduce is 1x-mode-capped on DVE; is_equal is 1x due to the stride-0
broadcast operand — dtype doesn't change their cycle count, so f32 is free).
In-DMAs issue from gpsimd (SWDGE) and out-DMAs from sync (HWDGE): separate
engine FIFOs, so a waiting out-DMA never head-of-line-blocks later in-DMAs.
All f32 values involved are exact (c <= 127 and 2^-7 scaling).
Sharding: pure data parallel over B*L rows, 8 cores, no collectives.
"""

import numpy as np

V = 64
P = 128
N_CORES = 8
B, L = 16, 8192
ROWS = B * L                      # 131072
ROWS_PER_CORE = ROWS // N_CORES   # 16384
SENTINEL = 100.0
EPS = 1.0 / 128.0

# rows per partition per chunk; chunk = [128, R*64] f32 = R*32KB
R = 16

_CACHE = {}


def _build_nc(rows_per_core: int, r: int, use_accum: bool = True):
    import concourse.bacc as bacc
    import concourse.mybir as mybir
    from concourse.bass import broadcast_tensor_aps
    from concourse.tile import TileContext

    f32 = mybir.dt.float32
    fd = r * V
    chunk_rows = P * r
    n_chunks = rows_per_core // chunk_rows
    assert rows_per_core % chunk_rows == 0

    # Bacc (not raw Bass): its compile() runs generate_event_semaphores(),
    # which legalizes multi-wait instructions for TRN2 (1 wait per instr).
    nc = bacc.Bacc("TRN2", target_bir_lowering=False, name="daf_onehot")
    x = nc.dram_tensor("x", [rows_per_core, V], f32, kind="ExternalInput")
    cmap = nc.dram_tensor("cmap", [P, V], f32, kind="ExternalInput")
    iota = nc.dram_tensor("iota", [P, V], f32, kind="ExternalInput")
    y = nc.dram_tensor("y", [rows_per_core, V], f32, kind="ExternalOutput")

    xv = x.rearrange("(c p r) v -> c p (r v)", p=P, r=r)
    yv = y.rearrange("(c p r) v -> c p (r v)", p=P, r=r)

    with TileContext(nc) as tc:
        with (
            tc.tile_pool(name="const", bufs=1) as constp,
            tc.tile_pool(name="io", bufs=n_chunks) as iop,
            tc.tile_pool(name="work", bufs=n_chunks) as workp,
        ):
            cmap_st = constp.tile([P, V], f32, tag="cmap_st")
            iota_st = constp.tile([P, V], f32, tag="iota_st")
            nc.sync.dma_start(cmap_st[:], cmap[:])
            nc.sync.dma_start(iota_st[:], iota[:])
            cmap_1 = cmap_st[:].rearrange("p (o v) -> p o v", o=1)
            iota_1 = iota_st[:].rearrange("p (o v) -> p o v", o=1)

            # Materialize iota replicated along r once; reused by every eq.
            iota_f = constp.tile([P, fd], f32, tag="iota_f")
            if3 = iota_f[:].rearrange("p (r v) -> p r v", v=V)
            io_b0, _ = broadcast_tensor_aps(iota_1, if3)
            nc.scalar.copy(if3, io_b0)

            for ci in range(n_chunks):
                prod = iop.tile([P, fd], f32, tag="prod")
                p3 = prod[:].rearrange("p (r v) -> p r v", v=V)
                cm_b, _ = broadcast_tensor_aps(cmap_1, p3)
                nc.scalar.copy(p3, cm_b)
                # SWDGE accum: prod = cmap_prefill + x (CCE add in the DMA)
                nc.gpsimd.dma_start(
                    prod[:], xv[ci], accum_op=mybir.AluOpType.add
                )

                c_t = workp.tile([P, r], f32, tag="c")
                nc.vector.tensor_reduce(
                    c_t[:], p3, axis=mybir.AxisListType.X, op=mybir.AluOpType.max
                )

                out_t = iop.tile([P, fd], f32, tag="out")
                o3 = out_t[:].rearrange("p (r v) -> p r v", v=V)
                c3 = c_t[:].rearrange("p (r one) -> p r one", one=1)
                c3_b, _ = broadcast_tensor_aps(c3, o3)
                nc.vector.tensor_tensor(
                    o3, if3, c3_b, op=mybir.AluOpType.is_equal
                )

                nc.sync.dma_start(yv[ci], out_t[:])

    # Bacc.finalize runs compile(): wait-splitting (generate_event_semaphores),
    # register allocation, nop fusion. run_bass_via_pjrt serializes nc.m as-is,
    # so this must happen here.
    nc.finalize()
    return nc


def _get_nc(rows_per_core=ROWS_PER_CORE, r=R, use_accum=True):
    key = (rows_per_core, r, use_accum)
    if key not in _CACHE:
        _CACHE[key] = _build_nc(rows_per_core, r, use_accum)
    return _CACHE[key]


def _host_cmap(W: np.ndarray, b: np.ndarray) -> np.ndarray:
    """64-entry map token -> output one-hot index (or sentinel for zero row)."""
    net = W.astype(np.float32) + b.astype(np.float32)[None, :]   # [V, 2V]
    loc_tok = np.argmax(net[:, :V], axis=1)                      # [V]
    scale_tok = np.argmax(net[:, V:], axis=1)                    # [V]
    t = (scale_tok * np.arange(V, dtype=np.int64) + loc_tok) % V
    return np.where(scale_tok == 0, SENTINEL, t.astype(np.float64)).astype(
        np.float32
    )


def _host_tables(W: np.ndarray, b: np.ndarray):
    cmap_eps = _host_cmap(W, b) * np.float32(EPS)                  # exact f32
    iota_eps = 1.0 + np.arange(V, dtype=np.float32) * np.float32(EPS)
    cmap_t = np.tile(cmap_eps.astype(np.float32)[None, :], (P, 1))
    iota_t = np.tile(iota_eps.astype(np.float32)[None, :], (P, 1))
    return cmap_t, iota_t


def kernel(inputs: np.ndarray, W: np.ndarray, b: np.ndarray) -> np.ndarray:
    from concourse import bass_utils

    x = np.ascontiguousarray(inputs.astype(np.float32, copy=False).reshape(ROWS, V))
    cmap_t, iota_t = _host_tables(W, b)

    nc = _get_nc()
    in_maps = [
        {
            "x": x[c * ROWS_PER_CORE : (c + 1) * ROWS_PER_CORE],
            "cmap": cmap_t,
            "iota": iota_t,
        }
        for c in range(N_CORES)
    ]
    res = bass_utils.run_bass_kernel_spmd(nc, in_maps, core_ids=list(range(N_CORES)))
    y = np.concatenate([r["y"] for r in res.results], axis=0)
    return y.reshape(inputs.shape).astype(inputs.dtype, copy=False)


# revision 3
# speedup vs baseline: 1.2476x; 1.2476x over previous
"""Trainium2 Bass kernel for nn_DiscreteAutoregressiveFlow (sampling, forward).

Math: `inputs` is an exact one-hot [B, L, V] tensor. For a row holding token v:
  net = W[v] + b                      (exact: one-hot @ W picks a row)
  loc = one_hot(argmax(net[:V]));  scale = one_hot(argmax(net[V:]))
  one_hot_multiply -> one-hot at (scale_tok*v) % V   (zero row if scale_tok==0)
  one_hot_add      -> one-hot at (scale_tok*v + loc_tok) % V
So out[row] = one_hot(cmap[v]) with a host-precomputed 64-entry map
(sentinel >= V encodes the zero row). The straight-through softmax residuals
and FFT noise in the reference are O(1e-7) and vanish in norm relative error.

Device pipeline per 128x(r*64) chunk (pure streaming, memory-bound):
  xt   = DMA-in (sync HWDGE)
  xb   = cast f32->bf16            (scalar ACT; enables DVE 2x add mode)
  prod = xb + cmap                 (DVE TT, bf16 2x)
  m    = reduce_max(prod, inner V) (DVE, 1x) = 1 + cmap[tok]/128, exact
  out  = is_equal(1 + iota/128, m) (DVE, 1x) -> exact 0.0/1.0 f32
  DMA-out (sync HWDGE)
Orchestration: ALL in-DMAs are issued on the sync FIFO BEFORE any out-DMA,
so an out-DMA waiting on compute can never head-of-line-block a later
in-DMA (the failure mode that serialized the interleaved version).
All f32/bf16 values involved are exact (c <= 127 and 2^-7 scaling).
Sharding: pure data parallel over B*L rows, 8 cores, no collectives.
"""

import numpy as np

V = 64
P = 128
N_CORES = 8
B, L = 16, 8192
ROWS = B * L                      # 131072
ROWS_PER_CORE = ROWS // N_CORES   # 16384
SENTINEL = 100.0
EPS = 1.0 / 128.0

# rows per partition per chunk; chunk = [128, R*64] f32 = R*32KB
R = 16

_CACHE = {}


def _build_nc(rows_per_core: int, r: int):
    import concourse.bacc as bacc
    import concourse.mybir as mybir
    from concourse.bass import broadcast_tensor_aps
    from concourse.tile import TileContext

    f32 = mybir.dt.float32
    bf16 = mybir.dt.bfloat16
    fd = r * V
    chunk_rows = P * r
    n_chunks = rows_per_core // chunk_rows
    assert rows_per_core % chunk_rows == 0

    # Bacc (not raw Bass): its compile() runs generate_event_semaphores(),
    # which legalizes multi-wait instructions for TRN2 (1 wait per instr).
    nc = bacc.Bacc("TRN2", target_bir_lowering=False, name="daf_onehot")
    x = nc.dram_tensor("x", [rows_per_core, V], f32, kind="ExternalInput")
    cmap = nc.dram_tensor("cmap", [P, V], f32, kind="ExternalInput")
    iota = nc.dram_tensor("iota", [P, V], f32, kind="ExternalInput")
    y = nc.dram_tensor("y", [rows_per_core, V], f32, kind="ExternalOutput")

    xv = x.rearrange("(c p r) v -> c p (r v)", p=P, r=r)
    yv = y.rearrange("(c p r) v -> c p (r v)", p=P, r=r)

    with TileContext(nc) as tc:
        with (
            tc.tile_pool(name="const", bufs=1) as constp,
            tc.tile_pool(name="io", bufs=n_chunks) as iop,
            tc.tile_pool(name="work", bufs=n_chunks) as workp,
        ):
            cmap_st = constp.tile([P, V], f32, tag="cmap_st")
            iota_st = constp.tile([P, V], f32, tag="iota_st")
            nc.sync.dma_start(cmap_st[:], cmap[:])
            nc.sync.dma_start(iota_st[:], iota[:])
            cmap_1 = cmap_st[:].rearrange("p (o v) -> p o v", o=1)
            iota_1 = iota_st[:].rearrange("p (o v) -> p o v", o=1)

            # Materialized bf16 broadcast tables (step-1 operands for DVE).
            cmap_f = constp.tile([P, fd], bf16, tag="cmap_f")
            cf3 = cmap_f[:].rearrange("p (r v) -> p r v", v=V)
            cm_b, _ = broadcast_tensor_aps(cmap_1, cf3)
            nc.scalar.copy(cf3, cm_b)
            iota_f = constp.tile([P, fd], bf16, tag="iota_f")
            if3 = iota_f[:].rearrange("p (r v) -> p r v", v=V)
            io_b0, _ = broadcast_tensor_aps(iota_1, if3)
            nc.scalar.copy(if3, io_b0)

            # Issue every in-DMA first: they sit ahead of all out-DMAs in the
            # sync FIFO, so they stream back-to-back from t=0.
            xts = []
            for ci in range(n_chunks):
                xt = iop.tile([P, fd], f32, tag="x")
                nc.sync.dma_start(xt[:], xv[ci])
                xts.append(xt)

            outs = []
            for ci in range(n_chunks):
                xt = xts[ci]
                xb_d = workp.tile([P, fd], bf16, tag="xb_d")
                nc.scalar.copy(xb_d[:], xt[:])

                prod = workp.tile([P, fd], bf16, tag="prod")
                p3 = prod[:].rearrange("p (r v) -> p r v", v=V)
                nc.vector.tensor_tensor(
                    prod[:], xb_d[:], cmap_f[:], op=mybir.AluOpType.add
                )

                c_t = workp.tile([P, r], f32, tag="c")
                nc.vector.tensor_reduce(
                    c_t[:], p3, axis=mybir.AxisListType.X, op=mybir.AluOpType.max
                )

                out_t = iop.tile([P, fd], f32, tag="out")
                o3 = out_t[:].rearrange("p (r v) -> p r v", v=V)
                c3 = c_t[:].rearrange("p (r one) -> p r one", one=1)
                c3_b, _ = broadcast_tensor_aps(c3, o3)
                nc.vector.tensor_tensor(o3, if3, c3_b, op=mybir.AluOpType.is_equal)
                outs.append(out_t)

            for ci in range(n_chunks):
                nc.sync.dma_start(yv[ci], outs[ci][:])

    # Bacc.finalize runs compile(): wait-splitting (generate_event_semaphores),
    # register allocation, nop fusion. run_bass_via_pjrt serializes nc.m as-is,
    # so this must happen here.
    nc.finalize()
    return nc


def _get_nc(rows_per_core=ROWS_PER_CORE, r=R):
    key = (rows_per_core, r)
    if key not in _CACHE:
        _CACHE[key] = _build_nc(rows_per_core, r)
    return _CACHE[key]


def _host_cmap(W: np.ndarray, b: np.ndarray) -> np.ndarray:
    """64-entry map token -> output one-hot index (or sentinel for zero row)."""
    net = W.astype(np.float32) + b.astype(np.float32)[None, :]   # [V, 2V]
    loc_tok = np.argmax(net[:, :V], axis=1)                      # [V]
    scale_tok = np.argmax(net[:, V:], axis=1)                    # [V]
    t = (scale_tok * np.arange(V, dtype=np.int64) + loc_tok) % V
    return np.where(scale_tok == 0, SENTINEL, t.astype(np.float64)).astype(
        np.float32
    )


def _host_tables(W: np.ndarray, b: np.ndarray):
    cmap_eps = _host_cmap(W, b) * np.float32(EPS)                  # exact f32
    iota_eps = 1.0 + np.arange(V, dtype=np.float32) * np.float32(EPS)
    cmap_t = np.tile(cmap_eps.astype(np.float32)[None, :], (P, 1))
    iota_t = np.tile(iota_eps.astype(np.float32)[None, :], (P, 1))
    return cmap_t, iota_t


def kernel(inputs: np.ndarray, W: np.ndarray, b: np.ndarray) -> np.ndarray:
    from concourse import bass_utils

    x = np.ascontiguousarray(inputs.astype(np.float32, copy=False).reshape(ROWS, V))
    cmap_t, iota_t = _host_tables(W, b)

    nc = _get_nc()
    in_maps = [
        {
            "x": x[c * ROWS_PER_CORE : (c + 1) * ROWS_PER_CORE],
            "cmap": cmap_t,
            "iota": iota_t,
        }
        for c in range(N_CORES)
    ]
    res = bass_utils.run_bass_kernel_spmd(nc, in_maps, core_ids=list(range(N_CORES)))
    y = np.concatenate([r["y"] for r in res.results], axis=0)
    return y.reshape(inputs.shape).astype(inputs.dtype, copy=False)
